# revision 1
# baseline (speedup 1.0000x reference)
"""Trainium2 Bass kernel for nn_BernConvLayer.

The reference computes, per graph b of B=8:
    Ahat = D^-1/2 (adj>0) D^-1/2
    BernConv(h) = sum_{k=0}^{K} relu(coe)[k] * C(K,k)/2^K * L^k (2I-L)^{K-k} h
with L = I - Ahat, K = 10, i.e. a degree-K polynomial p(Ahat) @ h with
monomial coefficients a_j derived from coe.  For the benchmark input
coe = ones(11), the binomial theorem gives
    sum_k C(K,k)/2^K (I-Ahat)^k (I+Ahat)^{K-k} = I        (exactly),
so every BernConv collapses to its input linear transform and the whole
module degenerates to a 4-matmul MLP per graph — `adj` contributes
nothing.  The coefficients a_j are computed exactly (integer polynomial
arithmetic); the collapsed path is taken only when a_1..a_K are exactly
zero, otherwise a full jax fallback reproduces the reference.

Kernel layout (collapsed path, one graph per NeuronCore, 8 cores):
  Everything runs in transposed space (features on SBUF partitions,
  nodes on the free dim) so no on-device transposes are needed:
    H0T = relu(W0T X^T + b0)  ... OUTT = WoutT BernT + bout
  The 2048-node free dim is streamed in 4 tiles of 512 (one PSUM bank).
  All matmuls use float32r (1 PE cycle/row vs 4 for strict fp32, fp32
  PSUM accumulation, ~1.7e-4 end-to-end rel err).  DMAs are issued in
  PE consumption order (w0/x chunked per K-tile for an early start,
  with the prefix's issue load spread across the SP, ACT and GPSIMD
  queues — issue costs ~1.2us of sequencer time per DMA), the OUT
  stage is software-pipelined three node-tiles behind the H stages so
  it never waits on wout's DMA, and stores ride the idle GPSIMD queue
  (the very last one rides ACT to skip SWDGE setup on the tail).
  Cost model timeline: 87.0us/core, all 336 matmuls at the full-speed
  213ns, mid-kernel gap-free; residual idle is the issue/BW-bound
  prefix and the fixed drain/barrier tail.
"""

import sys
from math import comb

import numpy as np

for _p in ("/opt/trn_rl_repo", "/root/.axon_site/_ro/trn_rl_repo"):
    if _p not in sys.path:
        sys.path.append(_p)

K = 10
B, N, H = 8, 2048, 768
HD = H // 3
P = 128
NTILE = 512
NT = N // NTILE


def _monomial_coeffs(coe: np.ndarray) -> np.ndarray:
    """Exact monomial coefficients a_j of p(s) = sum_k c_k (1-s)^k (1+s)^{K-k}.

    The integer coefficient matrix is built with Python ints, so the
    cancellation for coe=ones is exact (a = [1, 0, ..., 0])."""
    T = np.maximum(np.asarray(coe, np.float64), 0.0)
    a = np.zeros(K + 1)
    for k in range(K + 1):
        c_k = float(T[k]) * comb(K, k) / 2.0**K
        # integer coeffs of (1-s)^k (1+s)^{K-k}, increasing powers
        pa = [comb(k, i) * (-1) ** i for i in range(k + 1)]
        pb = [comb(K - k, i) for i in range(K - k + 1)]
        prod = [0] * (K + 1)
        for i, va in enumerate(pa):
            for j, vb in enumerate(pb):
                prod[i + j] += va * vb
        for j in range(K + 1):
            a[j] += c_k * prod[j]
    return a


# ---------------------------------------------------------------------------
# Bass kernel (collapsed MLP path)
# ---------------------------------------------------------------------------

_CACHE = {}


def _patch_drain_waits():
    """The axon-client walrus build rejects instructions with more sync
    waits than their ISA encoding holds ("Too many sync wait commands"):
    Drain (TPB_CTRL_NO_STRUCT) takes one, compute ops (e.g. Activation's
    S3D3_AC_STRUCT) fewer than Tile sometimes assigns.  Two fixes:
    (a) the kernel-tail drain's wait list is split across a chain of
        drains, each carrying a single wait;
    (b) every committed instruction with more than one wait gets the
        excess hoisted onto same-engine NOPs inserted immediately before
        it (same queue position, so semantics are unchanged).  This also
        covers DMACopy: it lowers to a PSEUDO_DMA TPB instruction on the
        issuing engine's queue, so queue order is preserved."""
    import concourse.mybir as mybir
    from concourse.tile import TileContext
    from concourse.vector_clock import ScopedClock
    import bass_rust

    if getattr(TileContext, "_drain_waits_patched", False):
        return

    _MAXW = 1

    _orig_commit = TileContext._commit_instruction

    def _split_commit_instruction(self, inst, lazy_reg_writes: bool = True):
        si = getattr(inst, "sync_info", None)
        eng = getattr(inst, "engine", None)
        if (
            si is not None
            and len(si.on_wait) > _MAXW
            and eng is not None
            and eng != mybir.EngineType.Unassigned
        ):
            waits = list(si.on_wait)
            while len(waits) > _MAXW:
                chunk, waits = waits[:_MAXW], waits[_MAXW:]
                nop = mybir.InstNoOp(
                    name=self.nc.get_next_instruction_name(),
                    sync_info=mybir.SyncInfo(on_wait=chunk, on_update=[]),
                    bass_nofuse=True,
                    engine=eng,
                )
                _orig_commit(self, nop, lazy_reg_writes=False)
            inst.sync_info = bass_rust.SyncInfo(
                on_wait=waits, on_update=list(si.on_update)
            )
        return _orig_commit(self, inst, lazy_reg_writes)

    TileContext._commit_instruction = _split_commit_instruction

    def _split_drain_and_barrier(self, tick_clock, wait_clock):
        drain_inst = self.nc.sync.drain()
        wait_clock.add_sem_waits(
            drain_inst.ins, ScopedClock({None: tick_clock.global_clock})
        )
        si = drain_inst.ins.sync_info
        if si is not None and len(si.on_wait) > 1:
            waits = list(si.on_wait)
            updates = list(si.on_update)
            drain_inst.ins.sync_info = bass_rust.SyncInfo(
                on_wait=waits[:1], on_update=[]
            )
            rest = waits[1:]
            while rest:
                chunk, rest = rest[:1], rest[1:]
                extra = self.nc.sync.drain()
                extra.ins.sync_info = bass_rust.SyncInfo(
                    on_wait=chunk, on_update=updates if not rest else []
                )

        self.nc.all_engine_barrier()
        assert self.sems is not None
        popped = self.nc._tile_sem_poison_stack.pop()
        assert popped is self._sem_poison
        self.nc.clear_and_free_semaphores(list(self.sems.allocated().values()))
        self.nc.all_engine_barrier()

    TileContext._drain_and_barrier = _split_drain_and_barrier
    TileContext._drain_waits_patched = True


def _build_mlp_bass(a0: float, use_f32r: bool = True):
    """Per-core MLP in transposed space.  DRAM params (all f32):
      xT   [768, 2048]   graph's node features, transposed
      w0   [768, 256], w1 [1024, 256], w2 [1280, 256], wout [768, 768]
      b0s, b1s, b2s [2, 128, 1]  (pre-scaled by a0),  bout [6, 128, 1]
      yT   [768, 2048]   output, transposed
    """
    import concourse.bass as bass
    import concourse.mybir as mybir
    from concourse.bass import ts
    from concourse.tile import TileContext

    _patch_drain_waits()

    f32 = mybir.dt.float32
    # float32r streams through the PE at 1 cycle/row (vs 4 for strict fp32)
    # with TF32-like reduced mantissa in the products; PSUM accumulation
    # stays fp32.
    mmdt = mybir.dt.float32r if use_f32r else f32
    AF = mybir.ActivationFunctionType


    nc = bass.Bass("TRN2", target_bir_lowering=False, debug=False)
    xT = nc.declare_dram_parameter("xT", [H, N], mmdt, isOutput=False)
    w0 = nc.declare_dram_parameter("w0", [H, HD], mmdt, isOutput=False)
    w1 = nc.declare_dram_parameter("w1", [H + HD, HD], mmdt, isOutput=False)
    w2 = nc.declare_dram_parameter("w2", [H + 2 * HD, HD], mmdt, isOutput=False)
    wout = nc.declare_dram_parameter("wout", [H, H], mmdt, isOutput=False)
    b0s = nc.declare_dram_parameter("b0s", [2, P, 1], f32, isOutput=False)
    b1s = nc.declare_dram_parameter("b1s", [2, P, 1], f32, isOutput=False)
    b2s = nc.declare_dram_parameter("b2s", [2, P, 1], f32, isOutput=False)
    bout = nc.declare_dram_parameter("bout", [6, P, 1], f32, isOutput=False)
    yT = nc.declare_dram_parameter("yT", [H, N], f32, isOutput=True)

    with TileContext(nc) as tc:
        with (
            tc.tile_pool(name="weights", bufs=1) as wpool,
            tc.tile_pool(name="xin", bufs=3) as xpool,
            tc.tile_pool(name="hid", bufs=2) as hpool,
            tc.tile_pool(name="bern", bufs=4) as bernpool,
            tc.tile_pool(name="yout", bufs=2) as ypool,
            tc.tile_pool(name="psum_h", bufs=4, space="PSUM") as psum_h,
            tc.tile_pool(name="psum_o", bufs=4, space="PSUM") as psum_o,
        ):
            xT_v = xT.rearrange("(t p) n -> p t n", p=P)
            yT_v = yT.rearrange("(t p) n -> p t n", p=P)
            w0v = w0.rearrange("(t p) m -> p t m", p=P)

            # Early delivery pacing: b0 + per-K-tile chunks of w0 and x
            # tile 0 so the PE starts ~5us in; w1 jumps the back half of
            # the x0 stream to balance delivery against consumption.
            def load_x(nn):
                xc = []
                for kk in range(6):
                    xk_t = xpool.tile(
                        [P, NTILE], mmdt, tag=f"x{kk}", name=f"x{kk}"
                    )
                    nc.sync.dma_start(
                        out=xk_t[:], in_=xT_v[:, kk, ts(nn, NTILE)]
                    )
                    xc.append(xk_t)
                return xc

            w0c = []
            x0c = []

            def chunk0(kk):
                wt = wpool.tile([P, HD], mmdt, name=f"w0c{kk}")
                nc.sync.dma_start(out=wt[:], in_=w0v[:, kk, :])
                w0c.append(wt)
                xk_t = xpool.tile([P, NTILE], mmdt, tag=f"x{kk}", name=f"x{kk}")
                nc.sync.dma_start(out=xk_t[:], in_=xT_v[:, kk, ts(0, NTILE)])
                x0c.append(xk_t)

            # The first w0/x0 chunk pair issues from the ACT queue: its
            # DGE setup runs in parallel with SP's, shaving the serial
            # issue chain off the very first matmul's critical path.
            wt0 = wpool.tile([P, HD], mmdt, name="w0c0")
            nc.scalar.dma_start(out=wt0[:], in_=w0v[:, 0, :])
            w0c.append(wt0)
            xk0 = xpool.tile([P, NTILE], mmdt, tag="x0", name="x0")
            nc.scalar.dma_start(out=xk0[:], in_=xT_v[:, 0, ts(0, NTILE)])
            x0c.append(xk0)
            b0_sb = wpool.tile([P, 2, 1], f32)
            nc.scalar.dma_start(out=b0_sb[:], in_=b0s.rearrange("m p o -> p m o"))
            for kk in range(1, 3):
                chunk0(kk)
            w1_sb = wpool.tile([P, 8, HD], mmdt)
            nc.sync.dma_start(out=w1_sb[:], in_=w1.rearrange("(t p) m -> p t m", p=P))
            # Chunks 3-5 issue from the idle GPSIMD queue: SP's issue
            # chain is ~1.26us per DMACopy and already deep; Pool's SWDGE
            # setup is slower per DMA but runs in parallel from t~1us.
            for kk in range(3, 6):
                wt = wpool.tile([P, HD], mmdt, name=f"w0c{kk}")
                nc.sync.dma_start(out=wt[:], in_=w0v[:, kk, :])
                w0c.append(wt)
                xk_t = xpool.tile([P, NTILE], mmdt, tag=f"x{kk}", name=f"x{kk}")
                nc.gpsimd.dma_start(out=xk_t[:], in_=xT_v[:, kk, ts(0, NTILE)])
                x0c.append(xk_t)
            b1_sb = wpool.tile([P, 2, 1], f32)
            nc.scalar.dma_start(out=b1_sb[:], in_=b1s.rearrange("m p o -> p m o"))
            b2_sb = wpool.tile([P, 2, 1], f32)
            nc.scalar.dma_start(out=b2_sb[:], in_=b2s.rearrange("m p o -> p m o"))
            w2_sb = wpool.tile([P, 10, HD], mmdt)
            nc.sync.dma_start(out=w2_sb[:], in_=w2.rearrange("(t p) m -> p t m", p=P))
            # x tiles 1 and 2 are prefetched BEFORE wout: the H stages of
            # those tiles fill the PE while wout (the largest weight) is
            # still streaming in, so OUT(0) never waits on it.
            x1c = load_x(1)
            x2c = load_x(2)
            bout_sb = wpool.tile([P, 6, 1], f32)
            nc.sync.dma_start(out=bout_sb[:], in_=bout.rearrange("m p o -> p m o"))
            wout_sb = wpool.tile([P, 6, H], mmdt)
            nc.sync.dma_start(
                out=wout_sb[:], in_=wout.rearrange("(t p) m -> p t m", p=P)
            )

            def h_stages(xc):
                """H0/H1/H2 for one node tile; K-outer so each chunk is
                consumed as it arrives."""

                def bern_conv(pairs, b_sb, out_t):
                    # m-outer: bank m=0 stops a full chain earlier than
                    # kk-outer would, so its relu (which the next stage's
                    # h-part matmuls wait on) pipelines ahead.
                    nk = len(pairs)
                    for m in range(2):
                        ps = psum_h.tile([P, NTILE], f32, tag="psh", name="psh")
                        for kk, (get_w, rhs_t) in enumerate(pairs):
                            nc.tensor.matmul(
                                ps[:], lhsT=get_w(m), rhs=rhs_t,
                                start=(kk == 0), stop=(kk == nk - 1))
                        nc.scalar.activation(
                            out_t[:, m, :], ps[:], AF.Relu,
                            bias=b_sb[:, m, :], scale=a0,
                        )

                def wslice(w_t, kk):
                    return lambda m: w_t[:, kk, ts(m, P)]

                h0 = hpool.tile([P, 2, NTILE], mmdt, tag="h0", name="h0")
                bern_conv(
                    [(lambda m, t=w0c[kk]: t[:, ts(m, P)], xc[kk])
                     for kk in range(6)],
                    b0_sb, h0)
                h1 = hpool.tile([P, 2, NTILE], mmdt, tag="h1", name="h1")
                bern_conv(
                    [(wslice(w1_sb, kk), xc[kk]) for kk in range(6)]
                    + [(wslice(w1_sb, 6 + j), h0[:, j, :]) for j in range(2)],
                    b1_sb, h1)
                h2 = hpool.tile([P, 2, NTILE], mmdt, tag="h2", name="h2")
                bern_conv(
                    [(wslice(w2_sb, kk), xc[kk]) for kk in range(6)]
                    + [(wslice(w2_sb, 6 + j), h0[:, j, :]) for j in range(2)]
                    + [(wslice(w2_sb, 8 + j), h1[:, j, :]) for j in range(2)],
                    b2_sb, h2)

                bern = bernpool.tile([P, 6, NTILE], mmdt, tag="bern", name="bern")
                for t, (ht, j) in enumerate(
                    [(h0, 0), (h0, 1), (h1, 0), (h1, 1), (h2, 0), (h2, 1)]
                ):
                    nc.vector.tensor_add(bern[:, t, :], xc[t][:], ht[:, j, :])
                return bern

            def out_stage(bern, nn, last=False):
                # Stores go out through the otherwise-idle GPSIMD queue so
                # their semaphore waits don't head-of-line-block x prefetch
                # issue on SP or the relu chain on ACT.
                for m in range(6):
                    ps = psum_o.tile([P, NTILE], f32, tag="pso", name="pso")
                    for kk in range(6):
                        nc.tensor.matmul(
                            ps[:], lhsT=wout_sb[:, kk, ts(m, P)],
                            rhs=bern[:, kk, :],
                            start=(kk == 0), stop=(kk == 5))
                    yt = ypool.tile([P, NTILE], f32, tag=f"yt{m}", name="yt")
                    nc.scalar.activation(
                        yt[:], ps[:], AF.Identity, bias=bout_sb[:, m, :]
                    )
                    # The very last store rides the ACT queue right behind
                    # its bias-add: HWDGE issue beats SWDGE setup on the
                    # kernel tail and nothing queues after it on ACT.
                    eng = nc.scalar if (last and m == 5) else nc.gpsimd
                    eng.dma_start(out=yT_v[:, m, ts(nn, NTILE)], in_=yt[:])

            # OUT is software-pipelined behind the H stages (depth 3 at
            # the start): tiles 0-2 do H work while wout streams in, so
            # OUT(0) starts with wout already resident.
            bern0 = h_stages(x0c)
            bern1 = h_stages(x1c)
            bern2 = h_stages(x2c)
            out_stage(bern0, 0)
            bern3 = h_stages(load_x(3))
            out_stage(bern1, 1)
            out_stage(bern2, 2)
            out_stage(bern3, 3, last=True)

    return nc


def _run_mlp(inputs: dict, a0: float, trace: bool = False, use_f32r: bool = True):
    from concourse.bass_utils import run_bass_kernel_spmd

    key = ("mlp", round(a0, 12), use_f32r)
    if key not in _CACHE:
        _CACHE[key] = _build_mlp_bass(a0, use_f32r)
    nc = _CACHE[key]

    f = np.float32
    x = np.asarray(inputs["x"], f)
    shared = {
        "w0": np.ascontiguousarray(inputs["W0"], f),
        "w1": np.ascontiguousarray(inputs["W1"], f),
        "w2": np.ascontiguousarray(inputs["W2"], f),
        "wout": np.ascontiguousarray(inputs["Wout"], f),
        "b0s": np.ascontiguousarray(a0 * np.asarray(inputs["b0"], f)).reshape(2, P, 1),
        "b1s": np.ascontiguousarray(a0 * np.asarray(inputs["b1"], f)).reshape(2, P, 1),
        "b2s": np.ascontiguousarray(a0 * np.asarray(inputs["b2"], f)).reshape(2, P, 1),
        "bout": np.ascontiguousarray(np.asarray(inputs["bout"], f)).reshape(6, P, 1),
    }
    in_maps = [
        {"xT": np.ascontiguousarray(x[i].T), **shared} for i in range(B)
    ]
    res = run_bass_kernel_spmd(nc, in_maps, list(range(B)), trace=trace)
    out = np.stack([res.results[i]["yT"].T for i in range(B)], axis=0)
    # Each run jits a fresh executable (new NEFF instance on every device);
    # drop them so repeated kernel() calls don't exhaust device resources.
    import jax

    jax.clear_caches()
    return np.ascontiguousarray(out, f), res


# ---------------------------------------------------------------------------
# General fallback (never taken for the benchmark input): full reference
# computation in jax.  Kept for correctness on arbitrary coe.
# ---------------------------------------------------------------------------


def _fallback_jax(inputs: dict) -> np.ndarray:
    import jax
    import jax.numpy as jnp

    def norm_adj(adj):
        A = (adj > 0).astype(adj.dtype)
        deg = A.sum(-1)
        dis = jnp.where(deg > 0, jax.lax.rsqrt(jnp.maximum(deg, 1e-12)), 0.0)
        return dis[..., :, None] * A * dis[..., None, :]

    def bern_conv(x, Ahat, coe, W, bvec):
        h = x @ W + bvec
        T = jax.nn.relu(coe)
        binom = jnp.asarray(
            [comb(K, k) / (2.0**K) for k in range(K + 1)], dtype=x.dtype
        )
        c = binom * T
        mm = lambda v: jnp.einsum("bij,bjh->bih", Ahat, v)
        tmp = [h]
        for _ in range(K):
            t = tmp[-1]
            tmp.append(t + mm(t))
        Lv = lambda v: v - mm(v)
        acc = c[K] * tmp[0]
        for i in range(K - 1, 0, -1):
            acc = Lv(acc) + c[i] * tmp[K - i]
        return c[0] * tmp[K] + Lv(acc)

    adj = jnp.asarray(inputs["adj"])
    x = jnp.asarray(inputs["x"])
    coe = jnp.asarray(inputs["coe"])
    Ahat = norm_adj(adj)
    h0 = jax.nn.relu(bern_conv(x, Ahat, coe, inputs["W0"], inputs["b0"]))
    h1 = jax.nn.relu(
        bern_conv(jnp.concatenate([x, h0], -1), Ahat, coe, inputs["W1"], inputs["b1"])
    )
    h2 = jax.nn.relu(
        bern_conv(
            jnp.concatenate([x, h0, h1], -1), Ahat, coe, inputs["W2"], inputs["b2"]
        )
    )
    bern = jnp.concatenate([h0, h1, h2], -1) + x
    out = bern @ jnp.asarray(inputs["Wout"]) + jnp.asarray(inputs["bout"])
    return np.asarray(out, np.float32)


def _collapsible(inputs: dict):
    if np.asarray(inputs["x"]).shape != (B, N, H):
        return None
    coe = np.asarray(inputs["coe"], np.float64)
    if coe.shape != (K + 1,):
        return None
    a = _monomial_coeffs(coe)
    if np.max(np.abs(a[1:])) <= 1e-12 * max(1.0, abs(a[0])):
        return float(a[0])
    return None


def kernel(**inputs) -> np.ndarray:
    a0 = _collapsible(inputs)
    if a0 is None:
        return _fallback_jax(inputs)
    out, _ = _run_mlp(inputs, a0)
    return out



# revision 3
# speedup vs baseline: 1.0781x; 1.0781x over previous
"""Trainium2 Bass kernel for nn_BernConvLayer.

The reference computes, per graph b of B=8:
    Ahat = D^-1/2 (adj>0) D^-1/2
    BernConv(h) = sum_{k=0}^{K} relu(coe)[k] * C(K,k)/2^K * L^k (2I-L)^{K-k} h
with L = I - Ahat, K = 10, i.e. a degree-K polynomial p(Ahat) @ h with
monomial coefficients a_j derived from coe.  For the benchmark input
coe = ones(11), the binomial theorem gives
    sum_k C(K,k)/2^K (I-Ahat)^k (I+Ahat)^{K-k} = I        (exactly),
so every BernConv collapses to its input linear transform and the whole
module degenerates to a 4-matmul MLP per graph — `adj` contributes
nothing.  The coefficients a_j are computed exactly (integer polynomial
arithmetic); the collapsed path is taken only when a_1..a_K are exactly
zero, otherwise a full jax fallback reproduces the reference.

Kernel layout (collapsed path, one graph per NeuronCore, 8 cores):
  Everything runs in transposed space (features on SBUF partitions,
  nodes on the free dim) so no on-device transposes are needed:
    H0T = relu(W0T X^T + b0)  ... OUTT = WoutT BernT + bout
  The 2048-node free dim is streamed in 4 tiles of 512 (one PSUM bank).
  All matmuls use float32r (1 PE cycle/row vs 4 for strict fp32, fp32
  PSUM accumulation, ~1.7e-4 end-to-end rel err).  DMAs are issued in
  PE consumption order (w0/x chunked per K-tile for an early start,
  with the prefix's issue load spread across the SP, ACT and GPSIMD
  queues — issue costs ~1.2us of sequencer time per DMA), the OUT
  stage is software-pipelined three node-tiles behind the H stages so
  it never waits on wout's DMA, and stores ride the idle GPSIMD queue
  (the very last one rides ACT to skip SWDGE setup on the tail).
  Cost model timeline: 87.0us/core, all 336 matmuls at the full-speed
  213ns, mid-kernel gap-free; residual idle is the issue/BW-bound
  prefix and the fixed drain/barrier tail.
"""

import sys
from math import comb

import numpy as np
import ml_dtypes

for _p in ("/opt/trn_rl_repo", "/root/.axon_site/_ro/trn_rl_repo"):
    if _p not in sys.path:
        sys.path.append(_p)

K = 10
B, N, H = 8, 2048, 768
HD = H // 3
P = 128
NTILE = 512
NT = N // NTILE

F8 = ml_dtypes.float8_e4m3
BF = ml_dtypes.bfloat16
S = 16.0  # common fp8 data scale (x, h, bern)
_DROP_XLO = False


def _monomial_coeffs(coe: np.ndarray) -> np.ndarray:
    """Exact monomial coefficients a_j of p(s) = sum_k c_k (1-s)^k (1+s)^{K-k}.

    The integer coefficient matrix is built with Python ints, so the
    cancellation for coe=ones is exact (a = [1, 0, ..., 0])."""
    T = np.maximum(np.asarray(coe, np.float64), 0.0)
    a = np.zeros(K + 1)
    for k in range(K + 1):
        c_k = float(T[k]) * comb(K, k) / 2.0**K
        # integer coeffs of (1-s)^k (1+s)^{K-k}, increasing powers
        pa = [comb(k, i) * (-1) ** i for i in range(k + 1)]
        pb = [comb(K - k, i) for i in range(K - k + 1)]
        prod = [0] * (K + 1)
        for i, va in enumerate(pa):
            for j, vb in enumerate(pb):
                prod[i + j] += va * vb
        for j in range(K + 1):
            a[j] += c_k * prod[j]
    return a


# ---------------------------------------------------------------------------
# Bass kernel (collapsed MLP path)
# ---------------------------------------------------------------------------

_CACHE = {}


def _patch_drain_waits():
    """The axon-client walrus build rejects instructions with more sync
    waits than their ISA encoding holds ("Too many sync wait commands"):
    Drain (TPB_CTRL_NO_STRUCT) takes one, compute ops (e.g. Activation's
    S3D3_AC_STRUCT) fewer than Tile sometimes assigns.  Two fixes:
    (a) the kernel-tail drain's wait list is split across a chain of
        drains, each carrying a single wait;
    (b) every committed instruction with more than one wait gets the
        excess hoisted onto same-engine NOPs inserted immediately before
        it (same queue position, so semantics are unchanged).  This also
        covers DMACopy: it lowers to a PSEUDO_DMA TPB instruction on the
        issuing engine's queue, so queue order is preserved."""
    import concourse.mybir as mybir
    from concourse.tile import TileContext
    from concourse.vector_clock import ScopedClock
    import bass_rust

    if getattr(TileContext, "_drain_waits_patched", False):
        return

    _MAXW = 1

    _orig_commit = TileContext._commit_instruction

    def _split_commit_instruction(self, inst, lazy_reg_writes: bool = True):
        si = getattr(inst, "sync_info", None)
        eng = getattr(inst, "engine", None)
        if (
            si is not None
            and len(si.on_wait) > _MAXW
            and eng is not None
            and eng != mybir.EngineType.Unassigned
        ):
            waits = list(si.on_wait)
            while len(waits) > _MAXW:
                chunk, waits = waits[:_MAXW], waits[_MAXW:]
                nop = mybir.InstNoOp(
                    name=self.nc.get_next_instruction_name(),
                    sync_info=mybir.SyncInfo(on_wait=chunk, on_update=[]),
                    bass_nofuse=True,
                    engine=eng,
                )
                _orig_commit(self, nop, lazy_reg_writes=False)
            inst.sync_info = bass_rust.SyncInfo(
                on_wait=waits, on_update=list(si.on_update)
            )
        return _orig_commit(self, inst, lazy_reg_writes)

    TileContext._commit_instruction = _split_commit_instruction

    def _split_drain_and_barrier(self, tick_clock, wait_clock):
        drain_inst = self.nc.sync.drain()
        wait_clock.add_sem_waits(
            drain_inst.ins, ScopedClock({None: tick_clock.global_clock})
        )
        si = drain_inst.ins.sync_info
        if si is not None and len(si.on_wait) > 1:
            waits = list(si.on_wait)
            updates = list(si.on_update)
            drain_inst.ins.sync_info = bass_rust.SyncInfo(
                on_wait=waits[:1], on_update=[]
            )
            rest = waits[1:]
            while rest:
                chunk, rest = rest[:1], rest[1:]
                extra = self.nc.sync.drain()
                extra.ins.sync_info = bass_rust.SyncInfo(
                    on_wait=chunk, on_update=updates if not rest else []
                )

        self.nc.all_engine_barrier()
        assert self.sems is not None
        popped = self.nc._tile_sem_poison_stack.pop()
        assert popped is self._sem_poison
        self.nc.clear_and_free_semaphores(list(self.sems.allocated().values()))
        self.nc.all_engine_barrier()

    TileContext._drain_and_barrier = _split_drain_and_barrier
    TileContext._drain_waits_patched = True


def _build_mlp_bass(a0: float, use_f32r: bool = True):
    """Per-core MLP in transposed space.  DRAM params (all f32):
      xT   [768, 2048]   graph's node features, transposed
      w0   [768, 256], w1 [1024, 256], w2 [1280, 256], wout [768, 768]
      b0s, b1s, b2s [2, 128, 1]  (pre-scaled by a0),  bout [6, 128, 1]
      yT   [768, 2048]   output, transposed
    """
    import concourse.bass as bass
    import concourse.mybir as mybir
    from concourse.bass import ts
    from concourse.tile import TileContext

    _patch_drain_waits()

    f32 = mybir.dt.float32
    # float32r streams through the PE at 1 cycle/row (vs 4 for strict fp32)
    # with TF32-like reduced mantissa in the products; PSUM accumulation
    # stays fp32.
    mmdt = mybir.dt.float32r if use_f32r else f32
    AF = mybir.ActivationFunctionType


    nc = bass.Bass("TRN2", target_bir_lowering=False, debug=False)
    xT = nc.declare_dram_parameter("xT", [H, N], mmdt, isOutput=False)
    w0 = nc.declare_dram_parameter("w0", [H, HD], mmdt, isOutput=False)
    w1 = nc.declare_dram_parameter("w1", [H + HD, HD], mmdt, isOutput=False)
    w2 = nc.declare_dram_parameter("w2", [H + 2 * HD, HD], mmdt, isOutput=False)
    wout = nc.declare_dram_parameter("wout", [H, H], mmdt, isOutput=False)
    b0s = nc.declare_dram_parameter("b0s", [2, P, 1], f32, isOutput=False)
    b1s = nc.declare_dram_parameter("b1s", [2, P, 1], f32, isOutput=False)
    b2s = nc.declare_dram_parameter("b2s", [2, P, 1], f32, isOutput=False)
    bout = nc.declare_dram_parameter("bout", [6, P, 1], f32, isOutput=False)
    yT = nc.declare_dram_parameter("yT", [H, N], f32, isOutput=True)

    with TileContext(nc) as tc:
        with (
            tc.tile_pool(name="weights", bufs=1) as wpool,
            tc.tile_pool(name="xin", bufs=3) as xpool,
            tc.tile_pool(name="hid", bufs=2) as hpool,
            tc.tile_pool(name="bern", bufs=4) as bernpool,
            tc.tile_pool(name="yout", bufs=2) as ypool,
            tc.tile_pool(name="psum_h", bufs=4, space="PSUM") as psum_h,
            tc.tile_pool(name="psum_o", bufs=4, space="PSUM") as psum_o,
        ):
            xT_v = xT.rearrange("(t p) n -> p t n", p=P)
            yT_v = yT.rearrange("(t p) n -> p t n", p=P)
            w0v = w0.rearrange("(t p) m -> p t m", p=P)

            # Early delivery pacing: b0 + per-K-tile chunks of w0 and x
            # tile 0 so the PE starts ~5us in; w1 jumps the back half of
            # the x0 stream to balance delivery against consumption.
            def load_x(nn):
                xc = []
                for kk in range(6):
                    xk_t = xpool.tile(
                        [P, NTILE], mmdt, tag=f"x{kk}", name=f"x{kk}"
                    )
                    nc.sync.dma_start(
                        out=xk_t[:], in_=xT_v[:, kk, ts(nn, NTILE)]
                    )
                    xc.append(xk_t)
                return xc

            w0c = []
            x0c = []

            def chunk0(kk):
                wt = wpool.tile([P, HD], mmdt, name=f"w0c{kk}")
                nc.sync.dma_start(out=wt[:], in_=w0v[:, kk, :])
                w0c.append(wt)
                xk_t = xpool.tile([P, NTILE], mmdt, tag=f"x{kk}", name=f"x{kk}")
                nc.sync.dma_start(out=xk_t[:], in_=xT_v[:, kk, ts(0, NTILE)])
                x0c.append(xk_t)

            # The first w0/x0 chunk pair issues from the ACT queue: its
            # DGE setup runs in parallel with SP's, shaving the serial
            # issue chain off the very first matmul's critical path.
            wt0 = wpool.tile([P, HD], mmdt, name="w0c0")
            nc.scalar.dma_start(out=wt0[:], in_=w0v[:, 0, :])
            w0c.append(wt0)
            xk0 = xpool.tile([P, NTILE], mmdt, tag="x0", name="x0")
            nc.scalar.dma_start(out=xk0[:], in_=xT_v[:, 0, ts(0, NTILE)])
            x0c.append(xk0)
            b0_sb = wpool.tile([P, 2, 1], f32)
            nc.scalar.dma_start(out=b0_sb[:], in_=b0s.rearrange("m p o -> p m o"))
            for kk in range(1, 3):
                chunk0(kk)
            w1_sb = wpool.tile([P, 8, HD], mmdt)
            nc.sync.dma_start(out=w1_sb[:], in_=w1.rearrange("(t p) m -> p t m", p=P))
            # Chunks 3-5 issue from the idle GPSIMD queue: SP's issue
            # chain is ~1.26us per DMACopy and already deep; Pool's SWDGE
            # setup is slower per DMA but runs in parallel from t~1us.
            for kk in range(3, 6):
                wt = wpool.tile([P, HD], mmdt, name=f"w0c{kk}")
                nc.sync.dma_start(out=wt[:], in_=w0v[:, kk, :])
                w0c.append(wt)
                xk_t = xpool.tile([P, NTILE], mmdt, tag=f"x{kk}", name=f"x{kk}")
                nc.gpsimd.dma_start(out=xk_t[:], in_=xT_v[:, kk, ts(0, NTILE)])
                x0c.append(xk_t)
            b1_sb = wpool.tile([P, 2, 1], f32)
            nc.scalar.dma_start(out=b1_sb[:], in_=b1s.rearrange("m p o -> p m o"))
            b2_sb = wpool.tile([P, 2, 1], f32)
            nc.scalar.dma_start(out=b2_sb[:], in_=b2s.rearrange("m p o -> p m o"))
            w2_sb = wpool.tile([P, 10, HD], mmdt)
            nc.sync.dma_start(out=w2_sb[:], in_=w2.rearrange("(t p) m -> p t m", p=P))
            # x tiles 1 and 2 are prefetched BEFORE wout: the H stages of
            # those tiles fill the PE while wout (the largest weight) is
            # still streaming in, so OUT(0) never waits on it.
            x1c = load_x(1)
            x2c = load_x(2)
            bout_sb = wpool.tile([P, 6, 1], f32)
            nc.sync.dma_start(out=bout_sb[:], in_=bout.rearrange("m p o -> p m o"))
            wout_sb = wpool.tile([P, 6, H], mmdt)
            nc.sync.dma_start(
                out=wout_sb[:], in_=wout.rearrange("(t p) m -> p t m", p=P)
            )

            def h_stages(xc):
                """H0/H1/H2 for one node tile; K-outer so each chunk is
                consumed as it arrives."""

                def bern_conv(pairs, b_sb, out_t):
                    # m-outer: bank m=0 stops a full chain earlier than
                    # kk-outer would, so its relu (which the next stage's
                    # h-part matmuls wait on) pipelines ahead.
                    nk = len(pairs)
                    for m in range(2):
                        ps = psum_h.tile([P, NTILE], f32, tag="psh", name="psh")
                        for kk, (get_w, rhs_t) in enumerate(pairs):
                            nc.tensor.matmul(
                                ps[:], lhsT=get_w(m), rhs=rhs_t,
                                start=(kk == 0), stop=(kk == nk - 1))
                        nc.scalar.activation(
                            out_t[:, m, :], ps[:], AF.Relu,
                            bias=b_sb[:, m, :], scale=a0,
                        )

                def wslice(w_t, kk):
                    return lambda m: w_t[:, kk, ts(m, P)]

                h0 = hpool.tile([P, 2, NTILE], mmdt, tag="h0", name="h0")
                bern_conv(
                    [(lambda m, t=w0c[kk]: t[:, ts(m, P)], xc[kk])
                     for kk in range(6)],
                    b0_sb, h0)
                h1 = hpool.tile([P, 2, NTILE], mmdt, tag="h1", name="h1")
                bern_conv(
                    [(wslice(w1_sb, kk), xc[kk]) for kk in range(6)]
                    + [(wslice(w1_sb, 6 + j), h0[:, j, :]) for j in range(2)],
                    b1_sb, h1)
                h2 = hpool.tile([P, 2, NTILE], mmdt, tag="h2", name="h2")
                bern_conv(
                    [(wslice(w2_sb, kk), xc[kk]) for kk in range(6)]
                    + [(wslice(w2_sb, 6 + j), h0[:, j, :]) for j in range(2)]
                    + [(wslice(w2_sb, 8 + j), h1[:, j, :]) for j in range(2)],
                    b2_sb, h2)

                bern = bernpool.tile([P, 6, NTILE], mmdt, tag="bern", name="bern")
                for t, (ht, j) in enumerate(
                    [(h0, 0), (h0, 1), (h1, 0), (h1, 1), (h2, 0), (h2, 1)]
                ):
                    nc.vector.tensor_add(bern[:, t, :], xc[t][:], ht[:, j, :])
                return bern

            def out_stage(bern, nn, last=False):
                # Stores go out through the otherwise-idle GPSIMD queue so
                # their semaphore waits don't head-of-line-block x prefetch
                # issue on SP or the relu chain on ACT.
                for m in range(6):
                    ps = psum_o.tile([P, NTILE], f32, tag="pso", name="pso")
                    for kk in range(6):
                        nc.tensor.matmul(
                            ps[:], lhsT=wout_sb[:, kk, ts(m, P)],
                            rhs=bern[:, kk, :],
                            start=(kk == 0), stop=(kk == 5))
                    yt = ypool.tile([P, NTILE], f32, tag=f"yt{m}", name="yt")
                    nc.scalar.activation(
                        yt[:], ps[:], AF.Identity, bias=bout_sb[:, m, :]
                    )
                    # The very last store rides the ACT queue right behind
                    # its bias-add: HWDGE issue beats SWDGE setup on the
                    # kernel tail and nothing queues after it on ACT.
                    eng = nc.scalar if (last and m == 5) else nc.gpsimd
                    eng.dma_start(out=yT_v[:, m, ts(nn, NTILE)], in_=yt[:])

            # OUT is software-pipelined behind the H stages (depth 3 at
            # the start): tiles 0-2 do H work while wout streams in, so
            # OUT(0) starts with wout already resident.
            bern0 = h_stages(x0c)
            bern1 = h_stages(x1c)
            bern2 = h_stages(x2c)
            out_stage(bern0, 0)
            bern3 = h_stages(load_x(3))
            out_stage(bern1, 1)
            out_stage(bern2, 2)
            out_stage(bern3, 3, last=True)

    return nc


def _run_mlp(inputs: dict, a0: float, trace: bool = False, use_f32r: bool = True):
    from concourse.bass_utils import run_bass_kernel_spmd

    key = ("mlp", round(a0, 12), use_f32r)
    if key not in _CACHE:
        _CACHE[key] = _build_mlp_bass(a0, use_f32r)
    nc = _CACHE[key]

    f = np.float32
    x = np.asarray(inputs["x"], f)
    shared = {
        "w0": np.ascontiguousarray(inputs["W0"], f),
        "w1": np.ascontiguousarray(inputs["W1"], f),
        "w2": np.ascontiguousarray(inputs["W2"], f),
        "wout": np.ascontiguousarray(inputs["Wout"], f),
        "b0s": np.ascontiguousarray(a0 * np.asarray(inputs["b0"], f)).reshape(2, P, 1),
        "b1s": np.ascontiguousarray(a0 * np.asarray(inputs["b1"], f)).reshape(2, P, 1),
        "b2s": np.ascontiguousarray(a0 * np.asarray(inputs["b2"], f)).reshape(2, P, 1),
        "bout": np.ascontiguousarray(np.asarray(inputs["bout"], f)).reshape(6, P, 1),
    }
    in_maps = [
        {"xT": np.ascontiguousarray(x[i].T), **shared} for i in range(B)
    ]
    res = run_bass_kernel_spmd(nc, in_maps, list(range(B)), trace=trace)
    out = np.stack([res.results[i]["yT"].T for i in range(B)], axis=0)
    # Each run jits a fresh executable (new NEFF instance on every device);
    # drop them so repeated kernel() calls don't exhaust device resources.
    import jax

    jax.clear_caches()
    return np.ascontiguousarray(out, f), res


# ---------------------------------------------------------------------------
# General fallback (never taken for the benchmark input): full reference
# computation in jax.  Kept for correctness on arbitrary coe.
# ---------------------------------------------------------------------------


def _fallback_jax(inputs: dict) -> np.ndarray:
    import jax
    import jax.numpy as jnp

    def norm_adj(adj):
        A = (adj > 0).astype(adj.dtype)
        deg = A.sum(-1)
        dis = jnp.where(deg > 0, jax.lax.rsqrt(jnp.maximum(deg, 1e-12)), 0.0)
        return dis[..., :, None] * A * dis[..., None, :]

    def bern_conv(x, Ahat, coe, W, bvec):
        h = x @ W + bvec
        T = jax.nn.relu(coe)
        binom = jnp.asarray(
            [comb(K, k) / (2.0**K) for k in range(K + 1)], dtype=x.dtype
        )
        c = binom * T
        mm = lambda v: jnp.einsum("bij,bjh->bih", Ahat, v)
        tmp = [h]
        for _ in range(K):
            t = tmp[-1]
            tmp.append(t + mm(t))
        Lv = lambda v: v - mm(v)
        acc = c[K] * tmp[0]
        for i in range(K - 1, 0, -1):
            acc = Lv(acc) + c[i] * tmp[K - i]
        return c[0] * tmp[K] + Lv(acc)

    adj = jnp.asarray(inputs["adj"])
    x = jnp.asarray(inputs["x"])
    coe = jnp.asarray(inputs["coe"])
    Ahat = norm_adj(adj)
    h0 = jax.nn.relu(bern_conv(x, Ahat, coe, inputs["W0"], inputs["b0"]))
    h1 = jax.nn.relu(
        bern_conv(jnp.concatenate([x, h0], -1), Ahat, coe, inputs["W1"], inputs["b1"])
    )
    h2 = jax.nn.relu(
        bern_conv(
            jnp.concatenate([x, h0, h1], -1), Ahat, coe, inputs["W2"], inputs["b2"]
        )
    )
    bern = jnp.concatenate([h0, h1, h2], -1) + x
    out = bern @ jnp.asarray(inputs["Wout"]) + jnp.asarray(inputs["bout"])
    return np.asarray(out, np.float32)


def _collapsible(inputs: dict):
    if np.asarray(inputs["x"]).shape != (B, N, H):
        return None
    coe = np.asarray(inputs["coe"], np.float64)
    if coe.shape != (K + 1,):
        return None
    a = _monomial_coeffs(coe)
    if np.max(np.abs(a[1:])) <= 1e-12 * max(1.0, abs(a[0])):
        return float(a[0])
    return None


# ---------------------------------------------------------------------------
# fp8 DoubleRow path (primary).  All matmuls run as fp8e4 DoubleRow chains
# (256-deep contraction per instruction, 0.5 PE cycles/row): every operand is
# a hi+lo fp8 pair at power-of-2 scales, combined as 3-term products
# (hi@Whi + hi@Wlo + lo@Whi) wherever full precision is needed.  x and
# weight pairs are split on the host; h stages emit bf16 (for the bern
# residual add) plus an fp8-hi cast (h0,h1 feed later stages single-fp8 -
# their output share is small).  bern = x + hcat forms in bf16 (x also
# shipped pre-tiled in bf16) and splits hi/lo on-device.  OUT accumulates
# (bern@Wout)*S*swo in PSUM; bf16 engine copies move it to SBUF and the host
# applies the exact power-of-2 descale plus bout.  DMAs are few and bundled:
# the shared HWDGE descriptor engine costs ~630ns per DMA regardless of
# size, and all transfers serialize on one DMA device.
# ---------------------------------------------------------------------------


def _wscale(w):
    return float(2.0 ** np.floor(np.log2(224.0 / max(np.abs(w).max(), 1e-30))))


def _split_fp8(a):
    hi = a.astype(F8)
    lo = (a - hi.astype(np.float32)).astype(F8)
    return hi, lo


def _pack_w(w, sw):
    """[K, M] f32 -> [P, 2*K//P, M] fp8, hi planes then lo planes."""
    kk, m = w.shape
    hi, lo = _split_fp8(np.asarray(w, np.float32) * sw)
    arr = lambda a: a.reshape(kk // P, P, m).transpose(1, 0, 2)
    return np.ascontiguousarray(np.concatenate([arr(hi), arr(lo)], axis=1))


def _pack_x(xi):
    """[N, H] f32 -> (xq [NT, P, 12, NTILE] fp8, xf [NT, P, 6, NTILE] bf16)."""
    xt = np.asarray(xi, np.float32).T * S  # [H, N]
    hi, lo = _split_fp8(xt)
    f16 = (hi.astype(np.float32) + lo.astype(np.float32)).astype(BF)
    arr = lambda a: a.reshape(6, P, NT, NTILE).transpose(2, 1, 0, 3)
    xq = np.ascontiguousarray(np.concatenate([arr(hi), arr(lo)], axis=2))
    xf = np.ascontiguousarray(arr(f16))
    return xq, xf


def _build_fp8_bass(a0, sw0, sw1, sw2, swo, drop_xlo=False):
    import concourse.bass as bass
    import concourse.mybir as mybir
    from concourse.bass import ts
    from concourse.tile import TileContext

    _patch_drain_waits()

    f32 = mybir.dt.float32
    bf16 = mybir.dt.bfloat16
    fp8 = mybir.dt.float8e4
    AF = mybir.ActivationFunctionType
    DR = mybir.MatmulPerfMode.DoubleRow

    nc = bass.Bass("TRN2", target_bir_lowering=False, debug=False)

    xq = nc.declare_dram_parameter("xq", [NT, P, 12, NTILE], fp8, isOutput=False)
    xfd = nc.declare_dram_parameter("xfd", [NT, P, 6, NTILE], bf16, isOutput=False)
    w0h = nc.declare_dram_parameter("w0h", [P, 6, HD], fp8, isOutput=False)
    w0l = nc.declare_dram_parameter("w0l", [P, 6, HD], fp8, isOutput=False)
    w1q = nc.declare_dram_parameter("w1q", [P, 16, HD], fp8, isOutput=False)
    w2q = nc.declare_dram_parameter("w2q", [P, 20, HD], fp8, isOutput=False)
    woq = nc.declare_dram_parameter("woq", [P, 12, H], fp8, isOutput=False)
    bq = nc.declare_dram_parameter("bq", [6, P, 1], f32, isOutput=False)
    yT = nc.declare_dram_parameter("yT", [P, 6, N], bf16, isOutput=True)

    with TileContext(nc) as tc:
        with (
            tc.tile_pool(name="weights", bufs=1) as wpool,
            tc.tile_pool(name="xin", bufs=1) as xpool,
            tc.tile_pool(name="xf", bufs=1) as xfpool,
            tc.tile_pool(name="hid", bufs=2) as hpool,
            tc.tile_pool(name="bern", bufs=4) as bernpool,
            tc.tile_pool(name="yout", bufs=2) as ypool,
            tc.tile_pool(name="psum_h", bufs=3, space="PSUM") as psum_h,
            tc.tile_pool(name="psum_o", bufs=5, space="PSUM") as psum_o,
        ):
            # ---- prefix loads, ordered by first consumption time ----
            # ACT queue: x tile 0 chunks (first chain inputs).  SP queue:
            # weights + later x/xf tiles.  The shared HWDGE device serves
            # roughly in issue order, so priority = program order.
            xq0 = xpool.tile([P, 12, NTILE], fp8, tag="xq0", name="xq0")
            nc.scalar.dma_start(out=xq0[:, 0:2, :], in_=xq[0, :, 0:2, :])
            w0h_sb = wpool.tile([P, 6, HD], fp8, name="w0h")
            nc.sync.dma_start(out=w0h_sb[:], in_=w0h[:])
            nc.scalar.dma_start(out=xq0[:, 2:6, :], in_=xq[0, :, 2:6, :])
            w0l_sb = wpool.tile([P, 6, HD], fp8, name="w0l")
            nc.sync.dma_start(out=w0l_sb[:], in_=w0l[:])
            b_sb = wpool.tile([P, 6, 1], f32, name="bq")
            nc.sync.dma_start(out=b_sb[:], in_=bq.rearrange("m p o -> p m o"))
            nc.scalar.dma_start(out=xq0[:, 6:12, :], in_=xq[0, :, 6:12, :])
            w1_sb = wpool.tile([P, 16, HD], fp8, name="w1q")
            nc.sync.dma_start(out=w1_sb[:], in_=w1q[:])
            w2_sb = wpool.tile([P, 20, HD], fp8, name="w2q")
            nc.sync.dma_start(out=w2_sb[:], in_=w2q[:])

            def xload(t):
                xt = xpool.tile([P, 12, NTILE], fp8, tag=f"xq{t}", name=f"xq{t}")
                nc.sync.dma_start(out=xt[:], in_=xq[t])
                return xt

            def xfload(t):
                f = xfpool.tile([P, 6, NTILE], bf16, tag=f"xf{t}", name=f"xf{t}")
                nc.sync.dma_start(out=f[:], in_=xfd[t])
                return f

            xq1 = xload(1)
            xq2 = xload(2)
            xf0 = xfload(0)
            wo_sb = wpool.tile([P, 12, H], fp8, name="woq")
            nc.sync.dma_start(out=wo_sb[:], in_=woq[:])
            xf1 = xfload(1)

            w_views = [
                (w0h_sb, w0l_sb, 6),
                (w1_sb[:, 0:8, :], w1_sb[:, 8:16, :], 8),
                (w2_sb[:, 0:10, :], w2_sb[:, 10:20, :], 10),
            ]

            def dr_chain(ps, pairs):
                n = len(pairs)
                for i, (lt, rt) in enumerate(pairs):
                    nc.tensor.matmul(
                        ps[:], lhsT=lt, rhs=rt,
                        start=(i == 0), stop=(i == n - 1), perf_mode=DR,
                    )

            def h_stages(t, xt):
                h_bf, h_hi = [], []
                for s, (wh, wl, nph) in enumerate(w_views):
                    hb = hpool.tile([P, 2, NTILE], bf16, tag=f"h{s}b", name=f"h{s}b_{t}")
                    sw = (sw0, sw1, sw2)[s]
                    for m in range(2):
                        ps = psum_h.tile([P, NTILE], f32, tag="psh", name="psh")
                        ms = ts(m, P)
                        pairs = []
                        for c in range(3):  # xhi @ Whi, xhi @ Wlo interleaved
                            pairs.append((wh[:, 2 * c : 2 * c + 2, ms], xt[:, 2 * c : 2 * c + 2, :]))
                            pairs.append((wl[:, 2 * c : 2 * c + 2, ms], xt[:, 2 * c : 2 * c + 2, :]))
                        if not drop_xlo:
                            for c in range(3):  # xlo @ Whi
                                pairs.append((wh[:, 2 * c : 2 * c + 2, ms], xt[:, 6 + 2 * c : 8 + 2 * c, :]))
                        for j in range(s):  # h_j_hi @ Whi, rows 768+256j
                            pl = 6 + 2 * j
                            pairs.append((wh[:, pl : pl + 2, ms], h_hi[j][:, 0:2, :]))
                        dr_chain(ps, pairs)
                        nc.scalar.activation(
                            hb[:, m, :], ps[:], AF.Relu,
                            bias=b_sb[:, 2 * s + m, :], scale=a0 / sw,
                        )
                    h_bf.append(hb)
                    if s < 2:
                        hh = hpool.tile([P, 2, NTILE], fp8, tag=f"h{s}q", name=f"h{s}q_{t}")
                        for m in range(2):
                            nc.scalar.copy(hh[:, m, :], hb[:, m, :])
                        h_hi.append(hh)
                return h_bf

            def bern_stage(t, xf, h_bf):
                bb = bernpool.tile([P, 6, NTILE], bf16, tag="bb", name=f"bb_{t}")
                bh = bernpool.tile([P, 6, NTILE], fp8, tag="bh", name=f"bh_{t}")
                bl = bernpool.tile([P, 6, NTILE], fp8, tag="bl", name=f"bl_{t}")
                for i in range(3):
                    sl = slice(2 * i, 2 * i + 2)
                    # DVE: bern = xf + h (bf16, 2x mode)
                    nc.vector.tensor_add(bb[:, sl, :], xf[:, sl, :], h_bf[i][:, 0:2, :])
                    # Pool: hi cast (SBUF-only op - Pool cannot touch PSUM)
                    nc.gpsimd.tensor_copy(bh[:, sl, :], bb[:, sl, :])
                    # DVE: lo = bern - hi
                    nc.vector.tensor_sub(bl[:, sl, :], bb[:, sl, :], bh[:, sl, :])
                return bh, bl

            def out_bank(t, m, bh, bl, yt, mm, tail=False):
                ps = psum_o.tile([P, NTILE], f32, tag="pso", name="pso")
                ms = ts(m, P)
                pairs = []
                for c in range(3):  # bhi @ Whi
                    pairs.append((wo_sb[:, 2 * c : 2 * c + 2, ms], bh[:, 2 * c : 2 * c + 2, :]))
                for c in range(3):  # bhi @ Wlo
                    pairs.append((wo_sb[:, 6 + 2 * c : 8 + 2 * c, ms], bh[:, 2 * c : 2 * c + 2, :]))
                for c in range(3):  # blo @ Whi
                    pairs.append((wo_sb[:, 2 * c : 2 * c + 2, ms], bl[:, 2 * c : 2 * c + 2, :]))
                dr_chain(ps, pairs)
                # psum -> sbuf bf16 copy: only ACT and DVE can read PSUM.
                if tail or m in (0, 3):
                    nc.scalar.copy(yt[:, mm, :], ps[:])
                else:
                    nc.vector.tensor_copy(yt[:, mm, :], ps[:])

            def out_stage(t, bh, bl, last=False):
                if last:
                    # tail: single-bank stores on the idle SP queue; the last
                    # transfer is half-size so the final sem fires sooner
                    for m in range(6):
                        yt = ypool.tile([P, 1, NTILE], bf16, tag=f"yl{m}", name="yl")
                        out_bank(t, m, bh, bl, yt, 0, tail=(m == 5))
                        nc.sync.dma_start(
                            out=yT[:, m : m + 1, ts(t, NTILE)], in_=yt[:]
                        )
                    return
                for u in range(3):  # store pair u covers banks 2u, 2u+1
                    yt = ypool.tile([P, 2, NTILE], bf16, tag=f"yt{u}", name="yt")
                    for mm in range(2):
                        out_bank(t, 2 * u + mm, bh, bl, yt, mm)
                    eng = (nc.sync, nc.scalar, nc.sync)[u]
                    eng.dma_start(
                        out=yT[:, 2 * u : 2 * u + 2, ts(t, NTILE)], in_=yt[:]
                    )

            # software pipeline (depth 3):
            # H(0) H(1) H(2) OUT(0) H(3) OUT(1) OUT(2) OUT(3)
            # bern(t) is issued after OUT(t-2) so each engine queue stays in
            # consumption order (a bern op ahead of OUT psum copies would
            # head-of-line-block psum recycling).
            h0 = h_stages(0, xq0)
            xq3 = xload(3)
            xf2 = xfload(2)
            bern0 = bern_stage(0, xf0, h0)
            h1 = h_stages(1, xq1)
            xf3 = xfload(3)
            bern1 = bern_stage(1, xf1, h1)
            h2 = h_stages(2, xq2)
            out_stage(0, *bern0)
            bern2 = bern_stage(2, xf2, h2)
            h3 = h_stages(3, xq3)
            out_stage(1, *bern1)
            bern3 = bern_stage(3, xf3, h3)
            out_stage(2, *bern2)
            out_stage(3, *bern3, last=True)

    return nc


def _run_fp8(inputs: dict, a0: float, trace: bool = False, drop_xlo: bool = False):
    from concourse.bass_utils import run_bass_kernel_spmd

    f = np.float32
    W0 = np.asarray(inputs["W0"], f)
    W1 = np.asarray(inputs["W1"], f)
    W2 = np.asarray(inputs["W2"], f)
    Wo = np.asarray(inputs["Wout"], f)
    sw0, sw1, sw2, swo = _wscale(W0), _wscale(W1), _wscale(W2), _wscale(Wo)

    key = ("fp8", round(a0, 12), sw0, sw1, sw2, swo, drop_xlo)
    if key not in _CACHE:
        _CACHE[key] = _build_fp8_bass(a0, sw0, sw1, sw2, swo, drop_xlo)
    nc = _CACHE[key]

    w0q = _pack_w(W0, sw0)  # [P, 12, HD]
    shared = {
        "w0h": np.ascontiguousarray(w0q[:, 0:6, :]),
        "w0l": np.ascontiguousarray(w0q[:, 6:12, :]),
        "w1q": _pack_w(W1, sw1),
        "w2q": _pack_w(W2, sw2),
        "woq": _pack_w(Wo, swo),
        "bq": np.ascontiguousarray(
            (a0 * S * np.stack(
                [np.asarray(inputs[f"b{s}"], f).reshape(2, P)[m]
                 for s in range(3) for m in range(2)]
            ))[:, :, None]
        ),
    }
    x = np.asarray(inputs["x"], f)
    in_maps = []
    for i in range(B):
        xqi, xfi = _pack_x(x[i])
        in_maps.append({"xq": xqi, "xfd": xfi, **shared})

    res = run_bass_kernel_spmd(nc, in_maps, list(range(B)), trace=trace)
    bout = np.asarray(inputs["bout"], f)
    descale = a0 / (S * swo)
    out = np.stack(
        [
            res.results[i]["yT"]
            .astype(f)
            .reshape(P, 6, N)
            .transpose(1, 0, 2)
            .reshape(H, N)
            .T
            * descale
            + bout
            for i in range(B)
        ],
        axis=0,
    )
    import jax

    jax.clear_caches()
    return np.ascontiguousarray(out, f), res


def _fp8_safe(inputs: dict, a0: float) -> bool:
    """Value-range guard for the fp8 path: every fp8-cast quantity must stay
    well inside e4m3's +-240 at scale S (checked on a node subsample)."""
    f = np.float32
    try:
        shapes = {
            "W0": (H, HD), "W1": (H + HD, HD), "W2": (H + 2 * HD, HD),
            "Wout": (H, H), "b0": (HD,), "b1": (HD,), "b2": (HD,),
            "bout": (H,),
        }
        for k, s in shapes.items():
            if np.asarray(inputs[k]).shape != s:
                return False
        x = np.asarray(inputs["x"], f)
        if not np.isfinite(x).all():
            return False
        if np.abs(x).max() * S > 232.0:
            return False
        xs = x[:, ::31, :].reshape(-1, H)
        relu = lambda v: np.maximum(v, 0.0)
        h0 = relu(a0 * (xs @ np.asarray(inputs["W0"], f) + np.asarray(inputs["b0"], f)))
        h1 = relu(a0 * (np.concatenate([xs, h0], 1) @ np.asarray(inputs["W1"], f)
                        + np.asarray(inputs["b1"], f)))
        h2 = relu(a0 * (np.concatenate([xs, h0, h1], 1) @ np.asarray(inputs["W2"], f)
                        + np.asarray(inputs["b2"], f)))
        bern = np.concatenate([h0, h1, h2], 1) + xs
        m = max(h0.max(initial=0), h1.max(initial=0), h2.max(initial=0),
                np.abs(bern).max())
        return bool(m * S * 2.0 <= 240.0)
    except Exception:
        return False


def kernel(**inputs) -> np.ndarray:
    a0 = _collapsible(inputs)
    if a0 is None:
        return _fallback_jax(inputs)
    if _fp8_safe(inputs, a0):
        out, _ = _run_fp8(inputs, a0, drop_xlo=_DROP_XLO)
        return out
    out, _ = _run_mlp(inputs, a0)
    return out

